# revision 1
# baseline (speedup 1.0000x reference)
"""Trainium2 Bass kernel for Swin-style cross-window attention.

Computation (per window b of 1024, N=64 tokens, C=512, H=16 heads, hd=32):
    qh = (q @ Wq.T + bq) * scale ; kh, vh likewise (no scale)
    attn = softmax(qh @ kh.T + rel_bias[h] + mask[b % 64])
    out  = (attn @ vh) @ Wp.T + bp

Sharding: data-parallel over the window axis across 8 NeuronCores
(128 windows / core).  Weights + bias tables replicated.

Per-core dataflow (blocks of 512 tokens = 8 windows):
    x natural [t,C] --PE transpose--> xT [C,t]
    qT/kT = W.T.T @ xT            (channels on partitions - heads contiguous)
    vh    = xT.T @ Wv.T           (tokens on partitions - natural layout)
    per window-pair (128 tokens on partitions):
        attn psum <- bias (eye-matmul init) ; += q_h.T x k_h.T (16 heads x 2
        windows packed on 32x32 PE sub-arrays)
        e = exp(attn) ; denom = rowsum ; e *= 1/denom
        eT = PE transpose(e) ; attnoutT = vh.T @ eT per head
    out = attnoutT.T @ Wp.T       (tokens on partitions, natural store)

Matmuls run as float32r (full-rate fp32 PE mode; operands must be written
as f32r by a compute op, per the BIR verifier); softmax/attn-values
optionally bf16 (E_BF16 knob).
"""

import functools
import os
import sys

import numpy as np

sys.path.insert(0, "/opt/trn_rl_repo")

import concourse.bass as bass
import concourse.mybir as mybir
import concourse.tile as tile
from concourse import bacc
from concourse.bass_utils import run_bass_kernel_spmd

# ---------------------------------------------------------------- constants
WH = WW = 8
N = 64                      # tokens per window
C = 512                     # channels
H = 16                      # heads
HD = C // H                 # 32
SCALE = HD ** -0.5
B_ = 1024                   # total windows
NW = 64                     # mask table size
NCORES = 8
WPC = B_ // NCORES          # 128 windows per core
TPC = WPC * N               # 8192 tokens per core
BLK = 512                   # tokens per block
NBLK = TPC // BLK           # 16
PAIRS = BLK // 128          # 4 window-pairs per block

FP32 = mybir.dt.float32
F32R = mybir.dt.float32r
BF16 = mybir.dt.bfloat16

E_BF16 = os.environ.get("KERNEL_E_BF16", "1") == "1"
TRACE = os.environ.get("KERNEL_TRACE", "0") == "1"


def _rel_pos_index():
    coords = np.stack(np.meshgrid(np.arange(WH), np.arange(WW), indexing="ij"))
    cf = coords.reshape(2, -1)
    rel = (cf[:, :, None] - cf[:, None, :]).transpose(1, 2, 0).astype(np.int64)
    rel[..., 0] += WH - 1
    rel[..., 1] += WW - 1
    rel[..., 0] *= 2 * WW - 1
    return rel.sum(-1)  # [N, N]


REL_IDX = _rel_pos_index()


# ---------------------------------------------------------------- bass module
@functools.lru_cache(maxsize=4)
def _build_nc(e_bf16: bool, use_mask: bool, use_bias_qk: bool, use_bias_vp: bool, nblk: int = NBLK, stage: int = 5):
    ED = BF16  # attention path dtype: f32r cannot target psum base!=0
    nc = bacc.Bacc("TRN2", target_bir_lowering=False)

    xq = nc.declare_dram_parameter("xq", [TPC, C], FP32, isOutput=False)
    xk = nc.declare_dram_parameter("xk", [TPC, C], FP32, isOutput=False)
    xv = nc.declare_dram_parameter("xv", [TPC, C], FP32, isOutput=False)
    wqt = nc.declare_dram_parameter("wqt", [C, C], FP32, isOutput=False)
    wkt = nc.declare_dram_parameter("wkt", [C, C], FP32, isOutput=False)
    wvt = nc.declare_dram_parameter("wvt", [C, C], FP32, isOutput=False)
    wpt = nc.declare_dram_parameter("wpt", [C, C], FP32, isOutput=False)
    # rel_bias [n, h, m]; with mask folded in it becomes per-window [w, n, h, m]
    if use_mask:
        cbias = nc.declare_dram_parameter("cbias", [NW, N, H, N], FP32, isOutput=False)
    else:
        bias_nhm = nc.declare_dram_parameter("bias_nhm", [N, H, N], FP32, isOutput=False)
    eye2 = nc.declare_dram_parameter("eye2", [N, 128], FP32, isOutput=False)
    id128 = nc.declare_dram_parameter("id128", [128, 128], FP32, isOutput=False)
    if e_bf16:
        ide = nc.declare_dram_parameter("ide", [128, 128], BF16, isOutput=False)
    if use_bias_qk:
        bqv = nc.declare_dram_parameter("bqv", [C], FP32, isOutput=False)  # pre-scaled
        bkv = nc.declare_dram_parameter("bkv", [C], FP32, isOutput=False)
    if use_bias_vp:
        bvv = nc.declare_dram_parameter("bvv", [128, C], FP32, isOutput=False)
        bpv = nc.declare_dram_parameter("bpv", [128, C], FP32, isOutput=False)
    out = nc.declare_dram_parameter("out", [TPC, C], FP32, isOutput=True)

    AF = mybir.ActivationFunctionType
    ALU = mybir.AluOpType

    from contextlib import ExitStack

    with tile.TileContext(nc) as tc, ExitStack() as stk:
        consts = stk.enter_context(tc.tile_pool(name="consts", bufs=1))
        # ---- constants: DMA in fp32, round-copy to f32r where needed.
        # Raw fp32 staging tiles live in a scratch pool released before the
        # main loop so their SBUF space is reclaimed.
        with tc.tile_pool(name="wtmp", bufs=1) as wtmp:
            w_sb = {}
            for nm, src in (("q", wqt), ("k", wkt), ("v", wvt), ("p", wpt)):
                raw = wtmp.tile([128, 4, C], FP32, tag=f"wraw{nm}", name=f"wraw{nm}")
                nc.sync.dma_start(raw[:], src.rearrange("(cs p) o -> p cs o", p=128))
                t = consts.tile([128, 4, C], F32R, tag=f"w{nm}", name=f"w{nm}_sb")
                nc.any.tensor_copy(out=t[:], in_=raw[:])
                w_sb[nm] = t
            id_sb = consts.tile([128, 128], FP32, tag="id128", name="id_sb")
            nc.sync.dma_start(id_sb[:], id128[:])
            if e_bf16:
                ide_sb = consts.tile([128, 128], BF16, tag="ide", name="ide_sb")
                nc.sync.dma_start(ide_sb[:], ide[:])
            else:
                ide_sb = consts.tile([128, 128], F32R, tag="ide", name="ide_sb")
                nc.any.tensor_copy(out=ide_sb[:], in_=id_sb[:])
            if not use_mask:
                bias_sb = consts.tile([128, H, N], FP32, tag="bias", name="bias_sb")
                nc.sync.dma_start(bias_sb[0:64], bias_nhm[:])
                nc.sync.dma_start(bias_sb[64:128], bias_nhm[:])
        kz_pool = stk.enter_context(tc.tile_pool(name="kz", bufs=1))
        kTzW = kz_pool.tile([128, 4, 4, BLK], BF16, tag="kTzW", name="kTzW")
        nc.vector.memset(kTzW[:], 0.0)
        with (
            tc.tile_pool(name="xn", bufs=2) as xn_pool,
            tc.tile_pool(name="xt", bufs=1) as xt_pool,
            tc.tile_pool(name="qk", bufs=1) as qk_pool,
            tc.tile_pool(name="vh", bufs=2) as vh_pool,
            tc.tile_pool(name="sm", bufs=4) as sm_pool,
            tc.tile_pool(name="ao", bufs=2) as ao_pool,
            tc.tile_pool(name="fin", bufs=2) as fin_pool,
            tc.tile_pool(name="psum", bufs=1, space="PSUM") as psum,
        ):
            if use_bias_qk:
                bq_sb = consts.tile([128, 4], FP32, tag="bq", name="bq_sb")
                nc.sync.dma_start(bq_sb[:], bqv.rearrange("(os p) -> p os", p=128))
                bk_sb = consts.tile([128, 4], FP32, tag="bk", name="bk_sb")
                nc.sync.dma_start(bk_sb[:], bkv.rearrange("(os p) -> p os", p=128))
            if use_bias_vp:
                bv_sb = consts.tile([128, C], FP32, tag="bv", name="bv_sb")
                nc.sync.dma_start(bv_sb[:], bvv[:])
                bp_sb = consts.tile([128, C], FP32, tag="bp", name="bp_sb")
                nc.sync.dma_start(bp_sb[:], bpv[:])

            # ---- per-block pipeline ---------------------------------------
            for b in range(nblk):
                t0 = b * BLK
                # load natural x tiles [p, ts, c]
                xn = {}
                for nm, src in (("q", xq), ("k", xk), ("v", xv)):
                    t = xn_pool.tile([128, 4, C], FP32, tag=f"xn{nm}", name=f"xn{nm}")
                    nc.sync.dma_start(
                        t[:], src[t0 : t0 + BLK].rearrange("(ts p) c -> p ts c", p=128)
                    )
                    xn[nm] = t
                # transpose to xT [p, cs, t] (f32r: the psum->sbuf copy rounds)
                xt = {}
                for nm in ("q", "k", "v"):
                    t = xt_pool.tile([128, 4, BLK], F32R, tag=f"xt{nm}", name=f"xt{nm}")
                    for ts in range(4):
                        ps = psum.tile([128, 4, 128], FP32, tag="tp", bufs=3,
                                       name="tp_ps")
                        for cs in range(4):
                            nc.tensor.transpose(
                                ps[:, cs, :], xn[nm][:, ts, cs * 128 : (cs + 1) * 128],
                                id_sb[:],
                            )
                        nc.any.tensor_copy(
                            out=t[:, :, ts * 128 : (ts + 1) * 128], in_=ps[:]
                        )
                    xt[nm] = t

                if stage == 1:
                    dbg = fin_pool.tile([128, 4, C], FP32, tag="fin", name="dbg1")
                    nc.any.tensor_copy(out=dbg[:], in_=xt["q"][:])
                    nc.sync.dma_start(
                        out[t0 : t0 + BLK].rearrange("(ts p) c -> p ts c", p=128), dbg[:]
                    )
                    continue

                # q/k projections -> qT/kT [p(c), os, t]
                qT = qk_pool.tile([128, 4, BLK], BF16, tag="qT", name="qT")
                for nm in ("q", "k"):
                    for os_ in range(4):
                        ps = psum.tile([128, BLK], FP32, tag="proj", bufs=2, name="proj_ps")
                        for cs in range(4):
                            nc.tensor.matmul(
                                ps[:],
                                w_sb[nm][:, cs, os_ * 128 : (os_ + 1) * 128],
                                xt[nm][:, cs, :],
                                start=(cs == 0),
                                stop=(cs == 3),
                            )
                        if nm == "q":
                            if use_bias_qk:
                                nc.vector.tensor_scalar(
                                    qT[:, os_, :], ps[:], SCALE,
                                    bq_sb[:, os_, None], ALU.mult, ALU.add,
                                )
                            else:
                                nc.scalar.activation(
                                    qT[:, os_, :], ps[:], AF.Copy, scale=SCALE
                                )
                        else:
                            # stripe each head-parity into its zero-padded copy
                            for c in range(4):
                                if use_bias_qk:
                                    nc.vector.tensor_scalar_add(
                                        kTzW[32 * c : 32 * c + 32, c, os_, :],
                                        ps[32 * c : 32 * c + 32, :],
                                        bk_sb[32 * c : 32 * c + 32, os_, None],
                                    )
                                else:
                                    nc.any.tensor_copy(
                                        out=kTzW[32 * c : 32 * c + 32, c, os_, :],
                                        in_=ps[32 * c : 32 * c + 32, :],
                                    )

                # v projection -> vh [p(t), ts, o] (natural), dtype ED
                vh = vh_pool.tile([128, 4, C], ED, tag="vh", name="vh")
                for ts in range(4):
                    ps = psum.tile([128, C], FP32, tag="proj", bufs=2, name="projv_ps")
                    for cs in range(4):
                        nc.tensor.matmul(
                            ps[:],
                            xt["v"][:, cs, ts * 128 : (ts + 1) * 128],
                            w_sb["v"][:, cs, :],
                            start=(cs == 0),
                            stop=(cs == 3),
                        )
                    if use_bias_vp:
                        nc.vector.tensor_tensor(
                            vh[:, ts, :], ps[:], bv_sb[:], ALU.add
                        )
                    else:
                        nc.any.tensor_copy(out=vh[:, ts, :], in_=ps[:])
                # vh with partition halves swapped (for head/window alignment)
                vhs = vh_pool.tile([128, 4, C], ED, tag="vhs", name="vhs")
                nc.sync.dma_start(vhs[0:64], vh[64:128])
                nc.sync.dma_start(vhs[64:128], vh[0:64])

                if stage == 2:
                    dbg = fin_pool.tile([128, 4, C], FP32, tag="fin", name="dbg2")
                    nc.any.tensor_copy(out=dbg[:], in_=qT[:])
                    nc.sync.dma_start(
                        out[t0 : t0 + BLK].rearrange("(ts p) c -> p ts c", p=128), dbg[:]
                    )
                    continue

                # attention output, transposed layout [p(c), os, t]
                aoT = ao_pool.tile([128, 4, BLK], F32R, tag="aoT", name="aoT")

                for p2 in range(PAIRS):
                    tp0 = p2 * 128
                    attn_psA = psum.tile([128, 8, N], FP32, tag="attnA", bufs=1,
                                         name="attn_psA")
                    attn_psB = psum.tile([128, 8, N], FP32, tag="attnB", bufs=1,
                                         name="attn_psB")
                    if use_mask:
                        cb = sm_pool.tile([128, H, N], FP32, tag="cb", name="cb", bufs=2)
                        w_abs = (b * 8 + p2 * 2) % NW
                        nc.sync.dma_start(
                            cb[:],
                            cbias[w_abs : w_abs + 2].rearrange("w n h m -> (w n) h m"),
                        )
                        bias_pair = cb
                    else:
                        bias_pair = bias_sb
                    # QK^T: each (head, window) writes its own psum region
                    for j in range(4):
                        aps = attn_psA if j < 2 else attn_psB
                        f0 = (4 * j) % 8
                        for w01 in range(2):
                            tq = tp0 + 64 * w01
                            nc.tensor.matmul(
                                aps[64 * w01 : 64 * w01 + 64, f0 : f0 + 4, :],
                                qT[:, j, tq : tq + 64],
                                kTzW[:, :, j, tq : tq + 64],
                                start=True,
                                stop=True,
                            )
                    # softmax (no max-subtraction: logits are O(1) by construction)
                    e = sm_pool.tile([128, H, N], ED, tag="e", name="e")
                    nc.vector.tensor_tensor(e[:, 0:8, :], attn_psA[:], bias_pair[:, 0:8, :], ALU.add)
                    nc.vector.tensor_tensor(e[:, 8:16, :], attn_psB[:], bias_pair[:, 8:16, :], ALU.add)
                    nc.scalar.activation(e[:], e[:], AF.Exp)
                    denom = sm_pool.tile([128, H], FP32, tag="denom", name="denom")
                    nc.vector.tensor_reduce(
                        denom[:], e[:], axis=mybir.AxisListType.X, op=ALU.add
                    )
                    recf = sm_pool.tile([128, H], FP32, tag="recf", name="recf")
                    nc.vector.reciprocal(recf[:], denom[:])
                    if e_bf16:
                        rec = sm_pool.tile([128, H], ED, tag="rec", name="rec")
                        nc.any.tensor_copy(out=rec[:], in_=recf[:])
                    else:
                        rec = recf
                    nc.vector.tensor_tensor(
                        e[:], e[:], rec[:, :, None].to_broadcast([128, H, N]), ALU.mult
                    )
                    if stage == 3:
                        continue
                    # transpose e -> eT [p(hp,m), ch, (w,n)]
                    eT = sm_pool.tile([128, 8, 128], ED, tag="eT", name="eT")
                    e_flat = e.rearrange("p h m -> p (h m)")
                    for g in range(2):
                        ps = psum.tile([128, 4, 128], ED, tag="tp", bufs=3, name="tpe_ps")
                        for cc in range(4):
                            ch = g * 4 + cc
                            nc.tensor.transpose(
                                ps[:, cc, :], e_flat[:, ch * 128 : (ch + 1) * 128],
                                ide_sb[:],
                            )
                        nc.any.tensor_copy(out=eT[:, g * 4 : g * 4 + 4, :], in_=ps[:])
                    # attn @ V  -> attnoutT
                    av_ps = psum.tile([128, 4, 128], FP32, tag="av", bufs=1, name="av_ps")
                    for h in range(H):
                        hp = h % 2
                        c0 = 32 * (h % 4)
                        for w01 in range(2):
                            vsrc = vh if hp == w01 else vhs
                            nc.tensor.matmul(
                                av_ps[c0 : c0 + 32, h // 4, 64 * w01 : 64 * w01 + 64],
                                vsrc[64 * hp : 64 * hp + 64, p2, 32 * h : 32 * h + 32],
                                eT[64 * hp : 64 * hp + 64, h // 2,
                                   64 * w01 : 64 * w01 + 64],
                                start=True,
                                stop=True,
                                tile_position=(64 * hp, c0),
                            )
                    nc.any.tensor_copy(out=aoT[:, :, tp0 : tp0 + 128], in_=av_ps[:])

                if stage == 3:
                    dbg = fin_pool.tile([128, 4, C], FP32, tag="fin", name="dbg3")
                    nc.any.tensor_copy(out=dbg[:], in_=vh[:])
                    nc.sync.dma_start(
                        out[t0 : t0 + BLK].rearrange("(ts p) c -> p ts c", p=128), dbg[:]
                    )
                    continue
                if stage == 4:
                    dbg = fin_pool.tile([128, 4, C], FP32, tag="fin", name="dbg4")
                    nc.any.tensor_copy(out=dbg[:], in_=aoT[:])
                    nc.sync.dma_start(
                        out[t0 : t0 + BLK].rearrange("(ts p) c -> p ts c", p=128), dbg[:]
                    )
                    continue

                # output projection -> natural [t, o] and store
                fin = fin_pool.tile([128, 4, C], FP32, tag="fin", name="fin")
                for ts in range(4):
                    ps = psum.tile([128, C], FP32, tag="proj", bufs=2, name="projf_ps")
                    for cs in range(4):
                        nc.tensor.matmul(
                            ps[:],
                            aoT[:, cs, ts * 128 : (ts + 1) * 128],
                            w_sb["p"][:, cs, :],
                            start=(cs == 0),
                            stop=(cs == 3),
                        )
                    if use_bias_vp:
                        nc.vector.tensor_tensor(
                            fin[:, ts, :], ps[:], bp_sb[:], ALU.add
                        )
                    else:
                        nc.any.tensor_copy(out=fin[:, ts, :], in_=ps[:])
                nc.sync.dma_start(
                    out[t0 : t0 + BLK].rearrange("(ts p) c -> p ts c", p=128), fin[:]
                )

    nc.compile()
    return nc


# ---------------------------------------------------------------- host entry
def kernel(q, k, v, mask, Wq, bq, Wk, bk, Wv, bv, Wp, bp, bias_table):
    q = np.ascontiguousarray(np.asarray(q, np.float32).reshape(B_ * N, C))
    k = np.ascontiguousarray(np.asarray(k, np.float32).reshape(B_ * N, C))
    v = np.ascontiguousarray(np.asarray(v, np.float32).reshape(B_ * N, C))
    mask = np.asarray(mask, np.float32)
    bias_table = np.asarray(bias_table, np.float32)

    wqt = np.ascontiguousarray(np.asarray(Wq, np.float32).T)
    wkt = np.ascontiguousarray(np.asarray(Wk, np.float32).T)
    wvt = np.ascontiguousarray(np.asarray(Wv, np.float32).T)
    wpt = np.ascontiguousarray(np.asarray(Wp, np.float32).T)

    rel = bias_table[REL_IDX.reshape(-1)].reshape(N, N, H)      # [n, m, h]
    bias_nhm = np.ascontiguousarray(rel.transpose(0, 2, 1))     # [n, h, m]

    use_mask = bool(np.any(mask))
    use_bias_qk = bool(np.any(bq) or np.any(bk))
    use_bias_vp = bool(np.any(bv) or np.any(bp))

    eye2 = np.ascontiguousarray(
        np.concatenate([np.eye(N, dtype=np.float32)] * 2, axis=1)
    )  # [64, 128]
    id128 = np.eye(128, dtype=np.float32)

    nc = _build_nc(E_BF16, use_mask, use_bias_qk, use_bias_vp)

    common = dict(wqt=wqt, wkt=wkt, wvt=wvt, wpt=wpt, eye2=eye2, id128=id128)
    if E_BF16:
        import ml_dtypes

        common["ide"] = np.eye(128).astype(ml_dtypes.bfloat16)
    if use_mask:
        # combined bias per absolute window index w (same for every core:
        # window (core*128 + wl) % 64 == wl % 64)
        cb = (
            mask[:, :, None, :].astype(np.float32)
            + bias_nhm[None, :, :, :]
        )  # [64, n, h, m]
        common["cbias"] = np.ascontiguousarray(cb)
    else:
        common["bias_nhm"] = bias_nhm
    if use_bias_qk:
        common["bqv"] = np.asarray(bq, np.float32) * np.float32(SCALE)
        common["bkv"] = np.asarray(bk, np.float32)
    if use_bias_vp:
        common["bvv"] = np.ascontiguousarray(
            np.broadcast_to(np.asarray(bv, np.float32), (128, C)))
        common["bpv"] = np.ascontiguousarray(
            np.broadcast_to(np.asarray(bp, np.float32), (128, C)))

    in_maps = []
    for c in range(NCORES):
        sl = slice(c * TPC, (c + 1) * TPC)
        m = dict(common)
        m["xq"] = q[sl]
        m["xk"] = k[sl]
        m["xv"] = v[sl]
        in_maps.append(m)

    res = run_bass_kernel_spmd(nc, in_maps, core_ids=list(range(NCORES)), trace=TRACE)
    out = np.concatenate([res.results[c]["out"] for c in range(NCORES)], axis=0)
    kernel.last_exec_time_ns = res.exec_time_ns
    return out.reshape(B_, N, C).astype(np.float32)


kernel.last_exec_time_ns = None

